# revision 21
# baseline (speedup 1.0000x reference)
"""Masked dot-product attention (B=16, Q=K=2048, D=64) on 8 Trainium2 cores.

out = softmax(Q K^T / sqrt(64) + mask(valid_lens)) V, reproducing
reference.py's masked_softmax to ~1e-2 absmax-relative.

Sharding / load balance
-----------------------
Work units are (batch, 512-wide q-block): 64 units whose cost is
nk(b) = ceil(valid_len[b]/128) k-tiles. Units are sorted by nk descending and
dealt round-robin into 8 slots x 8 cores, so every core runs the *same*
static SPMD program while the host packs each core's own data. Slots are
processed [2nd-smallest, big..small desc, smallest]: a small first slot gets
compute started early, and a tiny last slot minimizes the serial tail.

Per-core inputs arrive fp16-packed as three regions per slot:
  Q^T [128, 512]   d=64 rows duplicated into partitions 64-127 (PE row groups)
  K^T [128, ceil(w/2)*128]  k-tile ki lives ONLY in partition half (ki%2)*64
                   (no duplication: halves the K DMA bytes vs naive packing)
  V_aug [128, w*65]  [V | 1] with rows >= valid_len zeroed by the host

Device pipeline (fp16 PE streams; PSUM accumulates fp32)
--------------------------------------------------------
Per 3-k-tile group, software-pipelined across slot boundaries:
  PE : S^T[128k, 512q] per k-tile = matmul(lhsT=K^T-tile, rhs=Q^T),
       alternating PE row groups so consecutive matmuls stream concurrently
  exp: P = exp(S^T/8) over the 3-bank PSUM group. Most groups run on the
       Scalar/ACT engine (table exp). Every DVE_EVERY-th group instead runs
       on the Vector engine as a one-instruction Schraudolph approximation:
       i16 = rint(S * 0.125*1024/ln2 + (15<<10) - C); the int16 bit pattern
       IS fp16(exp(S/8)) to within +-3%. This offloads ~1/5 of the exp wall
       (the single biggest engine cost) from ACT to the otherwise-spare DVE.
       The softmax ratio uses the same approximated P in numerator and
       denominator, so only the ripple (not the scale) leaks into the output;
       measured absmax rel err ~1.2e-2 (budget 2e-2).
  PE : O^T_aug[65, 512q] += matmul(lhsT=V_aug-tile[128,65], rhs=P-slice)
Zeroed V_aug rows implement the mask exactly; the ones column accumulates the
softmax denominator for free (row 64). exp() without max-subtraction is safe
because logits ~ N(0,1).

Division epilogue (overlapped with later units' compute)
--------------------------------------------------------
Per unit: numerator copied PSUM->SBUF fp16 (releases the PSUM bank),
denominator row DMA'd out of PSUM, reciprocal via the single-pass
reciprocal_approx_fast (~51 ULP, 5x faster than the iterative divide),
broadcast across the 64 d-partitions via a DRAM-bounce DMA, multiply
(deferred one unit so no FIFO head-of-line blocks), fp16 DMA out.
The final unit reciprocates straight from PSUM and broadcasts on the
then-idle PE as an exact fp16 hi+lo pair, minimizing the tail chain.
The host transposes O^T -> O (and upcasts fp16 -> fp32) while unsharding.
"""

import sys

if "/opt/trn_rl_repo" not in sys.path:
    sys.path.insert(0, "/opt/trn_rl_repo")

import math

import numpy as np

import concourse.bass as bass
import concourse.mybir as mybir
import concourse.tile as tile
from concourse import bacc
from concourse.bass_utils import run_bass_kernel_spmd

B, Q, KLEN, D = 16, 2048, 2048, 64
QB = 512                      # q-block width per work unit
NCORES = 8
NSLOTS = (B * (Q // QB)) // NCORES   # 8 slots per core
KT = 128                      # k-tile height
GK = 3                        # k-tiles per exp group (3 PSUM banks)
DVE_EVERY = 6                 # every n-th exp group runs on DVE (Schraudolph)
F32 = mybir.dt.float32
F16 = mybir.dt.float16
I16 = mybir.dt.int16
NPF16 = np.float16

LN2 = math.log(2.0)
SCH_SCALE = 0.125 * 1024.0 / LN2          # folds the 1/sqrt(d) logit scale
SCH_BIAS = float((15 << 10) - 45)         # fp16 exponent bias, ripple-centered

LAST_RESULTS = None           # BassKernelResults of the most recent run

_cache: dict = {}


def _schedule(valid_lens):
    """Static work schedule from valid_lens (host-known at call time)."""
    nk = [max(1, -(-int(v) // KT)) for v in valid_lens]
    units = [(b, qb) for b in range(B) for qb in range(Q // QB)]
    units.sort(key=lambda u: (-nk[u[0]], u))
    slots_nk = [nk[units[NCORES * j][0]] for j in range(NSLOTS)]
    assign = [[units[NCORES * j + c] for j in range(NSLOTS)] for c in range(NCORES)]
    return nk, slots_nk, assign


def _slot_order(slots_nk):
    # Ascending: small slots first (small startup DMA, fast churn while the
    # input DMA stream runs ahead); the big final unit's compute hides every
    # mid-unit normalize chain, leaving only its own short chain as the tail.
    return sorted(range(NSLOTS), key=lambda j: slots_nk[j])


def _regions(slots_nk, order):
    """Per-slot packed regions (Q, K, V) in processing order: offsets into
    the per-core [128, TOT] fp16 data buffer."""
    offs = []
    x = 0
    for j in order:
        w = slots_nk[j]
        kw = -(-w // 2) * KT
        vw = w * 65
        offs.append((x, x + QB, x + QB + kw, x + QB + kw + vw))
        x += QB + kw + vw
    return offs, x


def _build(slots_nk):
    """Build + compile the single SPMD program for the given slot profile."""
    order = _slot_order(slots_nk)
    offs, tot = _regions(slots_nk, order)

    nc = bacc.Bacc()
    data_d = nc.dram_tensor("data", [2 * D, tot], F16, kind="ExternalInput").ap()
    out_d = nc.dram_tensor("out", [NSLOTS, D, QB], F16, kind="ExternalOutput").ap()

    with tile.TileContext(nc) as tc:
        with (
            tc.tile_pool(name="qpool", bufs=3) as qpool,
            tc.tile_pool(name="kpool", bufs=3) as kpool,
            tc.tile_pool(name="vpool", bufs=3) as vpool,
            tc.tile_pool(name="ppool", bufs=6) as ppool,
            tc.tile_pool(name="epool", bufs=4) as epool,
            tc.tile_pool(name="gpool", bufs=1) as gpool,
            tc.tile_pool(name="opool", bufs=2) as opool,
            tc.tile_pool(name="dpool", bufs=2, space="DRAM") as dpool,
            tc.tile_pool(name="psum_s", bufs=2, space="PSUM") as psum_s,
            tc.tile_pool(name="psum_o", bufs=2, space="PSUM") as psum_o,
        ):
            ones_sb = gpool.tile([1, D], F16, name="ones", tag="ones")
            nc.vector.memset(ones_sb, 1.0)

            slot_ctx = {}
            o_tiles = {}       # jpos -> numerator tile (SBUF fp32)
            rb_tiles = {}      # jpos -> broadcast reciprocal (SBUF fp32)
            pb_last = []       # last unit: broadcast reciprocal in PSUM
            pending_muls = []  # jpos of units whose normalize is deferred

            fetched = {}

            def fetch_slot(jpos):
                # issue a slot's input DMAs one slot ahead of its compute so
                # the PE never stalls on input data at a slot boundary
                if jpos >= NSLOTS or jpos in fetched:
                    return
                q0, k0, v0, e0 = offs[jpos]
                xq = qpool.tile([2 * D, QB], F16, tag="xq")
                nc.sync.dma_start(out=xq, in_=data_d[:, q0:k0])
                xk = kpool.tile([2 * D, v0 - k0], F16, tag="xk")
                nc.sync.dma_start(out=xk, in_=data_d[:, k0:v0])
                xv = vpool.tile([2 * D, e0 - v0], F16, tag="xv")
                nc.sync.dma_start(out=xv, in_=data_d[:, v0:e0])
                fetched[jpos] = (xq, xk, xv)

            def open_slot(jpos):
                fetch_slot(jpos)
                fetch_slot(jpos + 1)
                xq, xk, xv = fetched[jpos]
                po = psum_o.tile([65, QB], F32, tag="po")
                slot_ctx[jpos] = (xq, xk, xv, po, slots_nk[order[jpos]])

            def emit_mul(jpos, last=False):
                # normalize + write out unit jpos (inputs long ready)
                oa = o_tiles.pop(jpos)
                oo = opool.tile([D, QB], F16, tag="oo")
                rb = pb_last[0] if last else rb_tiles.pop(jpos)
                nc.vector.tensor_mul(oo, oa[0:D, :], rb)
                nc.gpsimd.dma_start(out=out_d[order[jpos]], in_=oo)

            def close_slot(jpos, last=False):
                _, _, _, po, _ = slot_ctx[jpos]
                if not last:
                    # numerator + denominator -> SBUF fp32 in ONE copy
                    # (frees the PSUM bank for the unit-after-next). The
                    # denominator row then hops to partition 0 via an
                    # SBUF->SBUF DMA (no engine time):
                    # reciprocal_approx_fast needs base-0 IEEE fp32 in SBUF
                    # (its BITWISE_NOT seed breaks on PSUM's accumulator
                    # format and on nonzero partition bases). A DRAM-bounce
                    # DMA broadcasts the reciprocal across 64 d-partitions.
                    oa = gpool.tile([D + 1, QB], F32, name=f"oa{jpos}",
                                    tag=f"oa{jpos}")
                    nc.vector.tensor_copy(oa, po)
                    o_tiles[jpos] = oa
                    dn = epool.tile([1, QB], F32, tag="dn")
                    nc.gpsimd.dma_start(out=dn, in_=oa[D:D + 1, :])
                    r_sb = epool.tile([1, QB], F32, tag="r")
                    nc.vector.reciprocal_approx_fast(r_sb, dn)
                    scratch = dpool.tile([1, QB], F32, tag="scr")
                    nc.gpsimd.dma_start(out=scratch, in_=r_sb)
                    rb = gpool.tile([D, QB], F32, name=f"rb{jpos}",
                                    tag=f"rb{jpos}")
                    bcast_src = bass.AP(
                        tensor=scratch.tensor,
                        offset=scratch.offset,
                        ap=[[0, D]] + [list(a) for a in scratch.ap],
                    )
                    nc.gpsimd.dma_start(out=rb, in_=bcast_src)
                    rb_tiles[jpos] = rb
                    pending_muls.append(jpos)
                    # keep two units in flight so the deferred multiply never
                    # head-of-line blocks the DVE queue on the bounce DMA
                    while len(pending_muls) > 2:
                        emit_mul(pending_muls.pop(0))
                    return
                # final unit: flush deferred work, then the shortest chain:
                # den row to partition 0 on DVE (lower latency than a DMA
                # hop), approx reciprocal, broadcast on the then-idle PE as
                # an exact fp16 hi+lo pair (r = hi + lo to 2^-22), numerator
                # copy on the then-idle ScalarE.
                while pending_muls:
                    emit_mul(pending_muls.pop(0))
                dn = epool.tile([1, QB], F32, tag="dn")
                nc.vector.tensor_copy(dn, po[D:D + 1, :])
                oa = gpool.tile([D + 1, QB], F32, name=f"oa{jpos}",
                                tag=f"oa{jpos}")
                nc.scalar.activation(oa, po,
                                     mybir.ActivationFunctionType.Copy)
                r_sb = epool.tile([1, QB], F32, tag="r")
                nc.vector.reciprocal_approx_fast(r_sb, dn)
                hi16 = epool.tile([1, QB], F16, tag="hi16")
                nc.scalar.activation(hi16, r_sb,
                                     mybir.ActivationFunctionType.Copy)
                lo16 = epool.tile([1, QB], F16, tag="lo16")
                nc.vector.tensor_sub(lo16, r_sb, hi16)
                pb = psum_s.tile([D, QB], F32, tag="ps")
                nc.tensor.matmul(pb, lhsT=ones_sb, rhs=hi16,
                                 start=True, stop=False)
                nc.tensor.matmul(pb, lhsT=ones_sb, rhs=lo16,
                                 start=False, stop=True)
                o_tiles[jpos] = oa
                pb_last.append(pb)
                emit_mul(jpos, last=True)

            # flat schedule of (slot position, k-tile group): the S->exp->O
            # software pipeline flows across slot boundaries without flushing
            sched = []
            for jpos in range(NSLOTS):
                w = slots_nk[order[jpos]]
                for g in range(-(-w // GK)):
                    sched.append((jpos, g))

            last_slot_groups = {i for i, (jp, _) in enumerate(sched)
                                if jp == NSLOTS - 1}

            pending = None      # (jpos, [(ki, ph, p_sb)...], closes_slot)
            for si, (jpos, g) in enumerate(sched):
                if g == 0:
                    open_slot(jpos)
                xq, xk, xv, po, w = slot_ctx[jpos]
                ks = [k for k in range(g * GK, min(g * GK + GK, w))]
                ww = len(ks) * QB
                ps = psum_s.tile([128, GK * QB], F32, tag="ps")
                for i, ki in enumerate(ks):
                    rg = (ki % 2) * D   # partition half holding this k-tile
                    nc.tensor.matmul(
                        ps[:, i * QB:(i + 1) * QB],
                        lhsT=xk[rg:rg + D, (ki // 2) * KT:(ki // 2 + 1) * KT],
                        rhs=xq[rg:rg + D, :],
                        start=True, stop=True,
                        tile_position=(rg, 0),
                    )
                if pending is not None:
                    pj, items, closes = pending
                    pva = slot_ctx[pj][2].rearrange("p (w c) -> p w c", c=65)
                    pw = slot_ctx[pj][4]
                    for ki, ph, p_prev in items:
                        nc.tensor.matmul(
                            slot_ctx[pj][3],
                            lhsT=pva[:, ki, :],
                            rhs=p_prev[:, ph * QB:(ph + 1) * QB],
                            start=(ki == 0), stop=(ki == pw - 1),
                        )
                    if closes:
                        close_slot(pj)
                p_sb = ppool.tile([128, GK * QB], F16, tag="p")
                if jpos >= NSLOTS - 4 and g % 2 == 1 and \
                        not (jpos == NSLOTS - 4 and g == 1):
                    # Schraudolph fp16-bit exp on DVE: one tensor_scalar op
                    nc.vector.tensor_scalar(
                        p_sb[:, :ww].bitcast(I16), ps[:, :ww],
                        SCH_SCALE, SCH_BIAS,
                        mybir.AluOpType.mult, mybir.AluOpType.add,
                    )
                else:
                    nc.scalar.activation(
                        p_sb[:, :ww], ps[:, :ww],
                        mybir.ActivationFunctionType.Exp, scale=0.125,
                    )
                pending = (jpos, [(ki, i, p_sb) for i, ki in enumerate(ks)],
                           g == -(-w // GK) - 1)
            pj, items, closes = pending
            pva = slot_ctx[pj][2].rearrange("p (w c) -> p w c", c=65)
            pw = slot_ctx[pj][4]
            for ki, ph, p_prev in items:
                nc.tensor.matmul(
                    slot_ctx[pj][3],
                    lhsT=pva[:, ki, :],
                    rhs=p_prev[:, ph * QB:(ph + 1) * QB],
                    start=(ki == 0), stop=(ki == pw - 1),
                )
            close_slot(pj, last=True)

    nc.compile()
    return nc


def _pack(queries, keys, values, valid_lens, slots_nk, assign):
    order = _slot_order(slots_nk)
    offs, tot = _regions(slots_nk, order)
    data = np.zeros((NCORES, 2 * D, tot), NPF16)
    for c in range(NCORES):
        for jpos, j in enumerate(order):
            b, qb = assign[c][j]
            w = slots_nk[j]
            vl = int(valid_lens[b])
            q0, k0, v0, e0 = offs[jpos]
            qt = queries[b, qb * QB:(qb + 1) * QB, :].T        # [D, QB]
            data[c, :D, q0:k0] = qt
            data[c, D:, q0:k0] = qt
            ktr = keys[b, :w * KT, :].T                       # [D, w*KT]
            for ki in range(w):
                rg = (ki % 2) * D
                col = k0 + (ki // 2) * KT
                data[c, rg:rg + D, col:col + KT] = \
                    ktr[:, ki * KT:(ki + 1) * KT]
            vv = np.zeros((w * KT, 65), np.float32)
            vv[:vl, :D] = values[b, :vl, :]
            vv[:vl, D] = 1.0
            # [128 partitions, w, 65] flattened on the free axis
            data[c, :, v0:e0] = (
                vv.reshape(w, KT, 65).transpose(1, 0, 2).reshape(KT, w * 65))
    return [{"data": data[c]} for c in range(NCORES)]


def kernel(queries, keys, values, valid_lens):
    global LAST_RESULTS
    queries = np.asarray(queries, dtype=np.float32)
    keys = np.asarray(keys, dtype=np.float32)
    values = np.asarray(values, dtype=np.float32)
    valid_lens = np.asarray(valid_lens)

    key = tuple(int(v) for v in valid_lens)
    if key not in _cache:
        nk, slots_nk, assign = _schedule(valid_lens)
        nc = _build(slots_nk)
        _cache[key] = (nc, slots_nk, assign)
    nc, slots_nk, assign = _cache[key]

    in_maps = _pack(queries, keys, values, valid_lens, slots_nk, assign)
    res = run_bass_kernel_spmd(nc, in_maps, list(range(NCORES)))
    LAST_RESULTS = res

    out = np.empty((B, Q, D), np.float32)
    for c in range(NCORES):
        oc = res.results[c]["out"]          # [NSLOTS, D, QB] fp16
        for j in range(NSLOTS):
            b, qb = assign[c][j]
            out[b, qb * QB:(qb + 1) * QB, :] = oc[j].T.astype(np.float32)
    return out
